# revision 1
# baseline (speedup 1.0000x reference)
"""MetaLSTMCell fused kernel for 8 Trainium2 NeuronCores.

Sharding: tensor-parallel over the 4*hidden output dims.
  - main LSTM: core k owns H-units [k*128,(k+1)*128) -> 512 rows of the
    4H=4096 gate dim (i/f/g/o slices kept per-shard).
  - meta LSTM: core k owns HH-units [k*32,(k+1)*32) -> 128 rows of 4HH=1024.
  - one AllGather assembles full meta_h_new [256,B] so each core computes
    the z embeddings locally; everything else is communication-free.

All activations/weights are pre-transposed on the host so every DMA is a
contiguous load with K (the contraction dim) on SBUF partitions. Matmuls
run as float32r (full PE rate at N>=256, near-fp32 accuracy).
"""

import numpy as np

import concourse.bacc as bacc
import concourse.mybir as mybir
from concourse.tile import TileContext
from concourse.bass_utils import run_bass_kernel_spmd

B, I, H, HH, E = 512, 1024, 1024, 256, 64
NCORES = 8
HS = H // NCORES          # 128 main hidden units per core
MS = HH // NCORES         # 32 meta hidden units per core
JW = 4 * HS               # 512 gate rows per core (main)
MW = 4 * MS               # 128 gate rows per core (meta)

F32 = mybir.dt.float32
F32R = mybir.dt.float32r
AF = mybir.ActivationFunctionType

_cache = {}


def _build():
    if "nc" in _cache:
        return _cache["nc"]

    nc = bacc.Bacc(None, target_bir_lowering=False, debug=False,
                   num_devices=NCORES)

    def dp(name, shape):
        return nc.declare_dram_parameter(name, list(shape), F32, isOutput=False)

    def do(name, shape):
        return nc.declare_dram_parameter(name, list(shape), F32, isOutput=True)

    # activations (transposed), shared across cores except the c-state slices
    d_xT = dp("xT", (I, B))          # input.T
    d_yT = dp("yT", (H, B))          # main_h.T
    d_mT = dp("mT", (HH, B))         # meta_h.T
    d_mcT = dp("mcT", (MS, B))       # meta_c.T   slice for this core
    d_ycT = dp("ycT", (HS, B))       # main_c.T   slice for this core
    # meta-LSTM weights (sharded rows, transposed)
    d_wihx = dp("wihx", (I, MW))     # weight_ih[:, :I][mrows].T
    d_wihh = dp("wihh", (H, MW))     # weight_ih[:, I:][mrows].T
    d_whh = dp("whh", (HH, MW))      # weight_hh[mrows].T
    # hyper-embedding weights (replicated, transposed)
    d_hzi = dp("hzi", (HH, E))
    d_hzH = dp("hzH", (HH, E))
    d_hzb = dp("hzb", (HH, E))
    # main-LSTM weights (sharded rows, transposed)
    d_wiH = dp("wiH", (I, JW))       # weight_iH[task][jrows].T
    d_wHH = dp("wHH", (H, JW))       # weight_HH[task][jrows].T
    d_dzi = dp("dzi", (E, JW))       # weight_dziH[jrows].T
    d_dzH = dp("dzH", (E, JW))       # weight_dzHH[jrows].T
    d_bz = dp("bz", (E, JW))         # weight_bzH[jrows].T
    # biases
    d_bhyp = dp("bhyp", (MS, 4))     # bias_hyper[mrows] as [MS,4] (col=gate)
    d_bmain = dp("bmain", (HS, 4))   # bias[jrows] as [HS,4] (col=gate)
    d_bi = dp("bi", (E, 1))
    d_bH = dp("bH", (E, 1))
    # outputs (transposed slices)
    d_omh = do("o_mh", (MS, B))
    d_omc = do("o_mc", (MS, B))
    d_oyh = do("o_yh", (HS, B))
    d_oyc = do("o_yc", (HS, B))

    KI, KH, KM = I // 128, H // 128, HH // 128   # 8, 8, 2 K-tiles

    with TileContext(nc) as tc:
        with (
            tc.tile_pool(name="wp", bufs=1) as wp,
            tc.tile_pool(name="sp", bufs=1) as sp,
            tc.tile_pool(name="dram", bufs=1, space="DRAM") as dram,
            tc.tile_pool(name="ps_bd", bufs=2, space="PSUM") as ps_bd,
        ):
            def load(pool, dram_ap, shape, name, rounded=True):
                t = pool.tile(list(shape), F32, name=name)
                if rounded:
                    nc.sync.dma_start(out=t[:].bitcast(F32R),
                                      in_=dram_ap.bitcast(F32R))
                else:
                    nc.sync.dma_start(out=t[:], in_=dram_ap)
                return t

            # ---- loads on the meta critical path first
            mT = [load(wp, d_mT[i * 128:(i + 1) * 128, :], (128, B), f"mT{i}")
                  for i in range(KM)]
            wihx = [load(wp, d_wihx[i * 128:(i + 1) * 128, :], (128, MW),
                         f"wihx{i}") for i in range(KI)]
            wihh = [load(wp, d_wihh[i * 128:(i + 1) * 128, :], (128, MW),
                         f"wihh{i}") for i in range(KH)]
            whh = [load(wp, d_whh[i * 128:(i + 1) * 128, :], (128, MW),
                        f"whh{i}") for i in range(KM)]
            xT = [load(wp, d_xT[i * 128:(i + 1) * 128, :], (128, B), f"xT{i}")
                  for i in range(KI)]
            yT = [load(wp, d_yT[i * 128:(i + 1) * 128, :], (128, B), f"yT{i}")
                  for i in range(KH)]
            bhyp = load(wp, d_bhyp[:, :], (MS, 4), "bhyp", rounded=False)
            mcT = load(wp, d_mcT[:, :], (MS, B), "mcT", rounded=False)
            # remaining weights
            hzi = [load(wp, d_hzi[i * 128:(i + 1) * 128, :], (128, E),
                        f"hzi{i}") for i in range(KM)]
            hzH = [load(wp, d_hzH[i * 128:(i + 1) * 128, :], (128, E),
                        f"hzH{i}") for i in range(KM)]
            hzb = [load(wp, d_hzb[i * 128:(i + 1) * 128, :], (128, E),
                        f"hzb{i}") for i in range(KM)]
            wiH = [load(wp, d_wiH[i * 128:(i + 1) * 128, :], (128, JW),
                        f"wiH{i}") for i in range(KI)]
            wHH = [load(wp, d_wHH[i * 128:(i + 1) * 128, :], (128, JW),
                        f"wHH{i}") for i in range(KH)]
            dzi = load(wp, d_dzi[:, :], (E, JW), "dzi")
            dzH = load(wp, d_dzH[:, :], (E, JW), "dzH")
            bz = load(wp, d_bz[:, :], (E, JW), "bz")
            bmain = load(wp, d_bmain[:, :], (HS, 4), "bmain", rounded=False)
            bi = load(wp, d_bi[:, :], (E, 1), "bi", rounded=False)
            bH = load(wp, d_bH[:, :], (E, 1), "bH", rounded=False)
            ycT = load(wp, d_ycT[:, :], (HS, B), "ycT", rounded=False)

            bounce = dram.tile([MS, B], F32, name="bounce")
            gathered = dram.tile([HH, B], F32, name="gathered",
                                 addr_space="Shared")

            def r(t):
                return t[:].bitcast(F32R)

            # ================= meta LSTM =================
            with tc.tile_pool(name="ps_meta", bufs=1, space="PSUM") as ps_m:
                pm = ps_m.tile([MW, B], F32, name="pm")
                n = 0
                for i in range(KI):
                    nc.tensor.matmul(pm[:], r(wihx[i]), r(xT[i]),
                                     start=(n == 0), stop=False)
                    n += 1
                for i in range(KH):
                    nc.tensor.matmul(pm[:], r(wihh[i]), r(yT[i]),
                                     start=False, stop=False)
                    n += 1
                for i in range(KM):
                    nc.tensor.matmul(pm[:], r(whh[i]), r(mT[i]),
                                     start=False, stop=(i == KM - 1))
                    n += 1

                # gates: rows [0:MS]=i, [MS:2MS]=f, [2MS:3MS]=g, [3MS:4MS]=o
                si = sp.tile([MS, B], F32, name="si")
                sf = sp.tile([MS, B], F32, name="sf")
                so = sp.tile([MS, B], F32, name="so")
                tg = sp.tile([MS, B], F32, name="tg")
                nc.scalar.activation(si[:], pm[0 * MS:1 * MS, :], AF.Sigmoid,
                                     bias=bhyp[:, 0:1])
                nc.scalar.activation(sf[:], pm[1 * MS:2 * MS, :], AF.Sigmoid,
                                     bias=bhyp[:, 1:2])
                nc.scalar.activation(so[:], pm[3 * MS:4 * MS, :], AF.Sigmoid,
                                     bias=bhyp[:, 3:4])
                nc.scalar.activation(tg[:], pm[2 * MS:3 * MS, :], AF.Tanh,
                                     bias=bhyp[:, 2:3])

            mc1 = sp.tile([MS, B], F32, name="mc1")
            mc2 = sp.tile([MS, B], F32, name="mc2")
            mcn = sp.tile([MS, B], F32, name="mcn")
            tcm = sp.tile([MS, B], F32, name="tcm")
            mhn = sp.tile([MS, B], F32, name="mhn")
            nc.vector.tensor_mul(mc1[:], sf[:], mcT[:])
            nc.vector.tensor_mul(mc2[:], si[:], tg[:])
            nc.vector.tensor_add(mcn[:], mc1[:], mc2[:])
            nc.scalar.activation(tcm[:], mcn[:], AF.Tanh)
            nc.vector.tensor_mul(mhn[:], so[:], tcm[:])
            nc.sync.dma_start(out=d_omc[:, :], in_=mcn[:])
            nc.sync.dma_start(out=d_omh[:, :], in_=mhn[:])
            nc.sync.dma_start(out=bounce[:], in_=mhn[:])

            # ================= AllGather meta_h_new =================
            nc.gpsimd.collective_compute(
                "AllGather",
                mybir.AluOpType.bypass,
                replica_groups=[list(range(NCORES))],
                ins=[bounce[:]],
                outs=[gathered[:]],
            )

            # ================= main LSTM: x/h heavy matmuls =================
            # these are independent of the collective and fill the PE while
            # the gather is in flight
            pb = []
            pd = []
            for j in range(4):
                pbj = ps_bd.tile([128, B], F32, name="pb", tag="pb")
                pdj = ps_bd.tile([128, B], F32, name="pd", tag="pd")
                for i in range(KI):
                    nc.tensor.matmul(
                        pbj[:], wiH[i][:, j * 128:(j + 1) * 128].bitcast(F32R),
                        r(xT[i]), start=(i == 0), stop=(i == KI - 1))
                for i in range(KH):
                    nc.tensor.matmul(
                        pdj[:], wHH[i][:, j * 128:(j + 1) * 128].bitcast(F32R),
                        r(yT[i]), start=(i == 0), stop=(i == KH - 1))
                pb.append(pbj)
                pd.append(pdj)

            # ================= z embeddings =================
            mhg = [None] * KM
            for i in range(KM):
                t = wp.tile([128, B], F32, name=f"mhg{i}")
                nc.sync.dma_start(
                    out=t[:].bitcast(F32R),
                    in_=gathered[i * 128:(i + 1) * 128, :].bitcast(F32R))
                mhg[i] = t

            zi = sp.tile([E, B], F32, name="zi")
            zH = sp.tile([E, B], F32, name="zH")
            zb = sp.tile([E, B], F32, name="zb")
            with tc.tile_pool(name="ps_z", bufs=1, space="PSUM") as ps_z:
                pzi = ps_z.tile([E, B], F32, name="pzi")
                pzH = ps_z.tile([E, B], F32, name="pzH")
                pzb = ps_z.tile([E, B], F32, name="pzb")
                for i in range(KM):
                    nc.tensor.matmul(pzi[:], r(hzi[i]), r(mhg[i]),
                                     start=(i == 0), stop=(i == KM - 1))
                for i in range(KM):
                    nc.tensor.matmul(pzH[:], r(hzH[i]), r(mhg[i]),
                                     start=(i == 0), stop=(i == KM - 1))
                for i in range(KM):
                    nc.tensor.matmul(pzb[:], r(hzb[i]), r(mhg[i]),
                                     start=(i == 0), stop=(i == KM - 1))
                nc.vector.tensor_scalar_add(zi[:].bitcast(F32R), pzi[:],
                                            bi[:, 0:1])
                nc.vector.tensor_scalar_add(zH[:].bitcast(F32R), pzH[:],
                                            bH[:, 0:1])
                nc.vector.tensor_copy(zb[:].bitcast(F32R), pzb[:])

            # ================= gate combine =================
            gate_fn = [AF.Sigmoid, AF.Sigmoid, AF.Tanh, AF.Sigmoid]
            gates = []
            with tc.tile_pool(name="ps_ace", bufs=1, space="PSUM") as ps_ace:
                for j in range(4):
                    pa = ps_ace.tile([128, B], F32, name="pa", tag="pa")
                    pc = ps_ace.tile([128, B], F32, name="pc", tag="pc")
                    pe = ps_ace.tile([128, B], F32, name="pe", tag="pe")
                    nc.tensor.matmul(
                        pa[:], dzi[:, j * 128:(j + 1) * 128].bitcast(F32R),
                        r(zi), start=True, stop=True)
                    nc.tensor.matmul(
                        pc[:], dzH[:, j * 128:(j + 1) * 128].bitcast(F32R),
                        r(zH), start=True, stop=True)
                    nc.tensor.matmul(
                        pe[:], bz[:, j * 128:(j + 1) * 128].bitcast(F32R),
                        r(zb), start=True, stop=True)
                    sa = sp.tile([128, B], F32, name="sa", tag="sa", bufs=2)
                    sc = sp.tile([128, B], F32, name="sc", tag="sc", bufs=2)
                    se = sp.tile([128, B], F32, name="se", tag="se", bufs=2)
                    nc.vector.tensor_copy(sa[:], pa[:])
                    nc.vector.tensor_copy(sc[:], pc[:])
                    nc.vector.tensor_scalar_add(se[:], pe[:],
                                                bmain[:, j:j + 1])
                    u = sp.tile([128, B], F32, name="u", tag="u", bufs=2)
                    v = sp.tile([128, B], F32, name="v", tag="v", bufs=2)
                    w = sp.tile([128, B], F32, name="w", tag="w", bufs=2)
                    pre = sp.tile([128, B], F32, name="pre", tag="pre", bufs=2)
                    nc.vector.tensor_mul(u[:], sa[:], pb[j][:])
                    nc.vector.tensor_mul(v[:], sc[:], pd[j][:])
                    nc.vector.tensor_add(w[:], u[:], v[:])
                    nc.vector.tensor_add(pre[:], w[:], se[:])
                    g = sp.tile([128, B], F32, name=f"gate{j}")
                    nc.scalar.activation(g[:], pre[:], gate_fn[j])
                    gates.append(g)

            # ================= main c/h =================
            gi, gf, gg, go = gates
            yc1 = sp.tile([HS, B], F32, name="yc1")
            yc2 = sp.tile([HS, B], F32, name="yc2")
            ycn = sp.tile([HS, B], F32, name="ycn")
            tcy = sp.tile([HS, B], F32, name="tcy")
            yhn = sp.tile([HS, B], F32, name="yhn")
            nc.vector.tensor_mul(yc1[:], gf[:], ycT[:])
            nc.vector.tensor_mul(yc2[:], gi[:], gg[:])
            nc.vector.tensor_add(ycn[:], yc1[:], yc2[:])
            nc.scalar.activation(tcy[:], ycn[:], AF.Tanh)
            nc.vector.tensor_mul(yhn[:], go[:], tcy[:])
            nc.sync.dma_start(out=d_oyc[:, :], in_=ycn[:])
            nc.sync.dma_start(out=d_oyh[:, :], in_=yhn[:])

    nc.compile()
    _cache["nc"] = nc
    return nc


def make_in_maps(inputs):
    f = lambda name: np.ascontiguousarray(np.asarray(inputs[name], np.float32))
    x = f("input")
    yh = f("main_h")
    yc = f("main_c")
    mh = f("meta_h")
    mc = f("meta_c")
    t = int(np.asarray(inputs["task_index"]))
    wiH = f("weight_iH")[t]          # [4H, I]
    wHH = f("weight_HH")[t]          # [4H, H]
    wih = f("weight_ih")             # [4HH, I+H]
    whh = f("weight_hh")             # [4HH, HH]
    hzi, hzH, hzb = f("weight_hzi"), f("weight_hzH"), f("weight_hzb")
    dzi, dzH, bz = f("weight_dziH"), f("weight_dzHH"), f("weight_bzH")
    b_i, b_H = f("bias_i"), f("bias_H")
    bias, bhyp = f("bias"), f("bias_hyper")

    C = np.ascontiguousarray
    xT = C(x.T)
    yT = C(yh.T)
    mT = C(mh.T)
    ycT = C(yc.T)
    mcT = C(mc.T)
    hziT, hzHT, hzbT = C(hzi.T), C(hzH.T), C(hzb.T)
    biT = C(b_i[:, None])
    bHT = C(b_H[:, None])

    in_maps = []
    for k in range(NCORES):
        mrows = np.concatenate([g * HH + np.arange(k * MS, (k + 1) * MS)
                                for g in range(4)])
        jrows = np.concatenate([g * H + np.arange(k * HS, (k + 1) * HS)
                                for g in range(4)])
        in_maps.append({
            "xT": xT, "yT": yT, "mT": mT,
            "mcT": C(mcT[k * MS:(k + 1) * MS]),
            "ycT": C(ycT[k * HS:(k + 1) * HS]),
            "wihx": C(wih[mrows, :I].T),
            "wihh": C(wih[mrows, I:].T),
            "whh": C(whh[mrows].T),
            "hzi": hziT, "hzH": hzHT, "hzb": hzbT,
            "wiH": C(wiH[jrows].T),
            "wHH": C(wHH[jrows].T),
            "dzi": C(dzi[jrows].T),
            "dzH": C(dzH[jrows].T),
            "bz": C(bz[jrows].T),
            "bhyp": C(bhyp[mrows].reshape(4, MS).T),
            "bmain": C(bias[jrows].reshape(4, HS).T),
            "bi": biT, "bH": bHT,
        })
    return in_maps


def assemble(results):
    mh = np.concatenate([results[k]["o_mh"] for k in range(NCORES)], axis=0).T
    mc = np.concatenate([results[k]["o_mc"] for k in range(NCORES)], axis=0).T
    yh = np.concatenate([results[k]["o_yh"] for k in range(NCORES)], axis=0).T
    yc = np.concatenate([results[k]["o_yc"] for k in range(NCORES)], axis=0).T
    return ((np.ascontiguousarray(yh), np.ascontiguousarray(yc)),
            (np.ascontiguousarray(mh), np.ascontiguousarray(mc)))


def kernel(**inputs):
    nc = _build()
    in_maps = make_in_maps(inputs)
    res = run_bass_kernel_spmd(nc, in_maps, list(range(NCORES)))
    return assemble(res.results)


# revision 2
# speedup vs baseline: 193.2729x; 193.2729x over previous
"""MetaLSTMCell fused kernel for 8 Trainium2 NeuronCores.

Sharding: tensor-parallel over the 4*hidden output dims.
  - main LSTM: core k owns H-units [k*128,(k+1)*128) -> 512 rows of the
    4H=4096 gate dim (i/f/g/o slices kept per-shard).
  - meta LSTM: core k owns HH-units [k*32,(k+1)*32) -> 128 rows of 4HH=1024.
  - one AllGather assembles full meta_h_new [256,B] so each core computes
    the z embeddings locally; everything else is communication-free.

All activations/weights are pre-transposed on the host so every DMA is a
contiguous load with K (the contraction dim) on SBUF partitions. Matmuls
run as float32r (full PE rate at N>=256, near-fp32 accuracy).
"""

import numpy as np

import concourse.bacc as bacc
import concourse.mybir as mybir
from concourse.tile import TileContext
from concourse.bass_utils import run_bass_kernel_spmd

B, I, H, HH, E = 512, 1024, 1024, 256, 64
NCORES = 8
HS = H // NCORES          # 128 main hidden units per core
MS = HH // NCORES         # 32 meta hidden units per core
JW = 4 * HS               # 512 gate rows per core (main)
MW = 4 * MS               # 128 gate rows per core (meta)

F32 = mybir.dt.float32
F32R = mybir.dt.float32r
AF = mybir.ActivationFunctionType

_cache = {}


def _declare_io(nc):
    def dp(name, shape):
        return nc.declare_dram_parameter(name, list(shape), F32, isOutput=False)

    def do(name, shape):
        return nc.declare_dram_parameter(name, list(shape), F32, isOutput=True)

    d = {}
    # activations (transposed); shared across cores except the c-state slices
    d["xT"] = dp("xT", (I, B))          # input.T
    d["yT"] = dp("yT", (H, B))          # main_h.T
    d["mT"] = dp("mT", (HH, B))         # meta_h.T
    d["mcT"] = dp("mcT", (MS, B))       # meta_c.T   slice for this core
    d["ycT"] = dp("ycT", (HS, B))       # main_c.T   slice for this core
    # meta-LSTM weights (sharded rows, transposed)
    d["wihx"] = dp("wihx", (I, MW))     # weight_ih[:, :I][mrows].T
    d["wihh"] = dp("wihh", (H, MW))     # weight_ih[:, I:][mrows].T
    d["whh"] = dp("whh", (HH, MW))      # weight_hh[mrows].T
    # hyper-embedding weights (replicated, transposed)
    d["hzi"] = dp("hzi", (HH, E))
    d["hzH"] = dp("hzH", (HH, E))
    d["hzb"] = dp("hzb", (HH, E))
    # main-LSTM weights (sharded rows, transposed)
    d["wiH"] = dp("wiH", (I, JW))       # weight_iH[task][jrows].T
    d["wHH"] = dp("wHH", (H, JW))       # weight_HH[task][jrows].T
    d["dzi"] = dp("dzi", (E, JW))       # weight_dziH[jrows].T
    d["dzH"] = dp("dzH", (E, JW))       # weight_dzHH[jrows].T
    d["bz"] = dp("bz", (E, JW))         # weight_bzH[jrows].T
    # biases
    d["bhyp"] = dp("bhyp", (MS, 4))     # bias_hyper[mrows] as [MS,4]
    d["bmain"] = dp("bmain", (HS, 4))   # bias[jrows] as [HS,4]
    d["bi"] = dp("bi", (E, 1))
    d["bH"] = dp("bH", (E, 1))
    # outputs (transposed slices)
    d["o_mh"] = do("o_mh", (MS, B))
    d["o_mc"] = do("o_mc", (MS, B))
    d["o_yh"] = do("o_yh", (HS, B))
    d["o_yc"] = do("o_yc", (HS, B))
    return d


def _emit_once(nc, tc, d):
    KI, KH, KM = I // 128, H // 128, HH // 128   # 8, 8, 2 K-tiles

    with (
        tc.tile_pool(name="wp", bufs=1) as wp,
        tc.tile_pool(name="sp", bufs=1) as sp,
        tc.tile_pool(name="dram", bufs=1, space="DRAM") as dram,
        tc.tile_pool(name="ps_bd", bufs=2, space="PSUM") as ps_bd,
    ):
        def load(pool, dram_ap, shape, name, rounded=True):
            t = pool.tile(list(shape), F32, name=name)
            if rounded:
                nc.sync.dma_start(out=t[:].bitcast(F32R),
                                  in_=dram_ap.bitcast(F32R))
            else:
                nc.sync.dma_start(out=t[:], in_=dram_ap)
            return t

        # ---- loads on the meta critical path first
        mT = [load(wp, d["mT"][i * 128:(i + 1) * 128, :], (128, B), f"mT{i}")
              for i in range(KM)]
        wihx = [load(wp, d["wihx"][i * 128:(i + 1) * 128, :], (128, MW),
                     f"wihx{i}") for i in range(KI)]
        wihh = [load(wp, d["wihh"][i * 128:(i + 1) * 128, :], (128, MW),
                     f"wihh{i}") for i in range(KH)]
        whh = [load(wp, d["whh"][i * 128:(i + 1) * 128, :], (128, MW),
                    f"whh{i}") for i in range(KM)]
        xT = [load(wp, d["xT"][i * 128:(i + 1) * 128, :], (128, B), f"xT{i}")
              for i in range(KI)]
        yT = [load(wp, d["yT"][i * 128:(i + 1) * 128, :], (128, B), f"yT{i}")
              for i in range(KH)]
        bhyp = load(wp, d["bhyp"][:, :], (MS, 4), "bhyp", rounded=False)
        mcT = load(wp, d["mcT"][:, :], (MS, B), "mcT", rounded=False)
        # remaining weights
        hzi = [load(wp, d["hzi"][i * 128:(i + 1) * 128, :], (128, E),
                    f"hzi{i}") for i in range(KM)]
        hzH = [load(wp, d["hzH"][i * 128:(i + 1) * 128, :], (128, E),
                    f"hzH{i}") for i in range(KM)]
        hzb = [load(wp, d["hzb"][i * 128:(i + 1) * 128, :], (128, E),
                    f"hzb{i}") for i in range(KM)]
        wiH = [load(wp, d["wiH"][i * 128:(i + 1) * 128, :], (128, JW),
                    f"wiH{i}") for i in range(KI)]
        wHH = [load(wp, d["wHH"][i * 128:(i + 1) * 128, :], (128, JW),
                    f"wHH{i}") for i in range(KH)]
        dzi = load(wp, d["dzi"][:, :], (E, JW), "dzi")
        dzH = load(wp, d["dzH"][:, :], (E, JW), "dzH")
        bz = load(wp, d["bz"][:, :], (E, JW), "bz")
        bmain = load(wp, d["bmain"][:, :], (HS, 4), "bmain", rounded=False)
        bi = load(wp, d["bi"][:, :], (E, 1), "bi", rounded=False)
        bH = load(wp, d["bH"][:, :], (E, 1), "bH", rounded=False)
        ycT = load(wp, d["ycT"][:, :], (HS, B), "ycT", rounded=False)

        bounce = dram.tile([MS, B], F32, name="bounce")
        gathered = dram.tile([HH, B], F32, name="gathered",
                             addr_space="Shared")

        def r(t):
            return t[:].bitcast(F32R)

        # ================= meta LSTM =================
        with tc.tile_pool(name="ps_meta", bufs=1, space="PSUM") as ps_m:
            pm = ps_m.tile([MW, B], F32, name="pm")
            for i in range(KI):
                nc.tensor.matmul(pm[:], r(wihx[i]), r(xT[i]),
                                 start=(i == 0), stop=False)
            for i in range(KH):
                nc.tensor.matmul(pm[:], r(wihh[i]), r(yT[i]),
                                 start=False, stop=False)
            for i in range(KM):
                nc.tensor.matmul(pm[:], r(whh[i]), r(mT[i]),
                                 start=False, stop=(i == KM - 1))

            # gate rows: [0:MS]=i, [MS:2MS]=f, [2MS:3MS]=g, [3MS:4MS]=o
            si = sp.tile([MS, B], F32, name="si")
            sf = sp.tile([MS, B], F32, name="sf")
            so = sp.tile([MS, B], F32, name="so")
            tg = sp.tile([MS, B], F32, name="tg")
            nc.scalar.activation(si[:], pm[0 * MS:1 * MS, :], AF.Sigmoid,
                                 bias=bhyp[:, 0:1])
            nc.scalar.activation(sf[:], pm[1 * MS:2 * MS, :], AF.Sigmoid,
                                 bias=bhyp[:, 1:2])
            nc.scalar.activation(so[:], pm[3 * MS:4 * MS, :], AF.Sigmoid,
                                 bias=bhyp[:, 3:4])
            nc.scalar.activation(tg[:], pm[2 * MS:3 * MS, :], AF.Tanh,
                                 bias=bhyp[:, 2:3])

        mc1 = sp.tile([MS, B], F32, name="mc1")
        mc2 = sp.tile([MS, B], F32, name="mc2")
        mcn = sp.tile([MS, B], F32, name="mcn")
        tcm = sp.tile([MS, B], F32, name="tcm")
        mhn = sp.tile([MS, B], F32, name="mhn")
        nc.vector.tensor_mul(mc1[:], sf[:], mcT[:])
        nc.vector.tensor_mul(mc2[:], si[:], tg[:])
        nc.vector.tensor_add(mcn[:], mc1[:], mc2[:])
        nc.scalar.activation(tcm[:], mcn[:], AF.Tanh)
        nc.vector.tensor_mul(mhn[:], so[:], tcm[:])
        nc.sync.dma_start(out=d["o_mc"][:, :], in_=mcn[:])
        nc.sync.dma_start(out=d["o_mh"][:, :], in_=mhn[:])
        nc.sync.dma_start(out=bounce[:], in_=mhn[:])

        # ================= AllGather meta_h_new =================
        nc.gpsimd.collective_compute(
            "AllGather",
            mybir.AluOpType.bypass,
            replica_groups=[list(range(NCORES))],
            ins=[bounce[:]],
            outs=[gathered[:]],
        )

        # ============ main LSTM: x/h heavy matmuls ============
        # independent of the collective; fill the PE during the gather
        pb = []
        pd = []
        for j in range(4):
            pbj = ps_bd.tile([128, B], F32, name="pb", tag="pb")
            pdj = ps_bd.tile([128, B], F32, name="pd", tag="pd")
            for i in range(KI):
                nc.tensor.matmul(
                    pbj[:], wiH[i][:, j * 128:(j + 1) * 128].bitcast(F32R),
                    r(xT[i]), start=(i == 0), stop=(i == KI - 1))
            for i in range(KH):
                nc.tensor.matmul(
                    pdj[:], wHH[i][:, j * 128:(j + 1) * 128].bitcast(F32R),
                    r(yT[i]), start=(i == 0), stop=(i == KH - 1))
            pb.append(pbj)
            pd.append(pdj)

        # ================= z embeddings =================
        mhg = [None] * KM
        for i in range(KM):
            t = wp.tile([128, B], F32, name=f"mhg{i}")
            nc.sync.dma_start(
                out=t[:].bitcast(F32R),
                in_=gathered[i * 128:(i + 1) * 128, :].bitcast(F32R))
            mhg[i] = t

        zi = sp.tile([E, B], F32, name="zi")
        zH = sp.tile([E, B], F32, name="zH")
        zb = sp.tile([E, B], F32, name="zb")
        with tc.tile_pool(name="ps_z", bufs=1, space="PSUM") as ps_z:
            pzi = ps_z.tile([E, B], F32, name="pzi")
            pzH = ps_z.tile([E, B], F32, name="pzH")
            pzb = ps_z.tile([E, B], F32, name="pzb")
            for i in range(KM):
                nc.tensor.matmul(pzi[:], r(hzi[i]), r(mhg[i]),
                                 start=(i == 0), stop=(i == KM - 1))
            for i in range(KM):
                nc.tensor.matmul(pzH[:], r(hzH[i]), r(mhg[i]),
                                 start=(i == 0), stop=(i == KM - 1))
            for i in range(KM):
                nc.tensor.matmul(pzb[:], r(hzb[i]), r(mhg[i]),
                                 start=(i == 0), stop=(i == KM - 1))
            nc.vector.tensor_scalar_add(zi[:].bitcast(F32R), pzi[:],
                                        bi[:, 0:1])
            nc.vector.tensor_scalar_add(zH[:].bitcast(F32R), pzH[:],
                                        bH[:, 0:1])
            nc.vector.tensor_copy(zb[:].bitcast(F32R), pzb[:])

        # ================= gate combine =================
        gate_fn = [AF.Sigmoid, AF.Sigmoid, AF.Tanh, AF.Sigmoid]
        gates = []
        with tc.tile_pool(name="ps_ace", bufs=1, space="PSUM") as ps_ace:
            for j in range(4):
                pa = ps_ace.tile([128, B], F32, name="pa", tag="pa")
                pc = ps_ace.tile([128, B], F32, name="pc", tag="pc")
                pe = ps_ace.tile([128, B], F32, name="pe", tag="pe")
                nc.tensor.matmul(
                    pa[:], dzi[:, j * 128:(j + 1) * 128].bitcast(F32R),
                    r(zi), start=True, stop=True)
                nc.tensor.matmul(
                    pc[:], dzH[:, j * 128:(j + 1) * 128].bitcast(F32R),
                    r(zH), start=True, stop=True)
                nc.tensor.matmul(
                    pe[:], bz[:, j * 128:(j + 1) * 128].bitcast(F32R),
                    r(zb), start=True, stop=True)
                sa = sp.tile([128, B], F32, name="sa", tag="sa", bufs=2)
                sc = sp.tile([128, B], F32, name="sc", tag="sc", bufs=2)
                se = sp.tile([128, B], F32, name="se", tag="se", bufs=2)
                nc.vector.tensor_copy(sa[:], pa[:])
                nc.vector.tensor_copy(sc[:], pc[:])
                nc.vector.tensor_scalar_add(se[:], pe[:], bmain[:, j:j + 1])
                u = sp.tile([128, B], F32, name="u", tag="u", bufs=2)
                v = sp.tile([128, B], F32, name="v", tag="v", bufs=2)
                w = sp.tile([128, B], F32, name="w", tag="w", bufs=2)
                pre = sp.tile([128, B], F32, name="pre", tag="pre", bufs=2)
                nc.vector.tensor_mul(u[:], sa[:], pb[j][:])
                nc.vector.tensor_mul(v[:], sc[:], pd[j][:])
                nc.vector.tensor_add(w[:], u[:], v[:])
                nc.vector.tensor_add(pre[:], w[:], se[:])
                g = sp.tile([128, B], F32, name=f"gate{j}")
                nc.scalar.activation(g[:], pre[:], gate_fn[j])
                gates.append(g)

        # ================= main c/h =================
        gi, gf, gg, go = gates
        yc1 = sp.tile([HS, B], F32, name="yc1")
        yc2 = sp.tile([HS, B], F32, name="yc2")
        ycn = sp.tile([HS, B], F32, name="ycn")
        tcy = sp.tile([HS, B], F32, name="tcy")
        yhn = sp.tile([HS, B], F32, name="yhn")
        nc.vector.tensor_mul(yc1[:], gf[:], ycT[:])
        nc.vector.tensor_mul(yc2[:], gi[:], gg[:])
        nc.vector.tensor_add(ycn[:], yc1[:], yc2[:])
        nc.scalar.activation(tcy[:], ycn[:], AF.Tanh)
        nc.vector.tensor_mul(yhn[:], go[:], tcy[:])
        nc.sync.dma_start(out=d["o_yc"][:, :], in_=ycn[:])
        nc.sync.dma_start(out=d["o_yh"][:, :], in_=yhn[:])


def build_program(reps=1):
    """Build + compile the SPMD program. reps>1 repeats the whole body
    (used by the timing harness to cancel per-launch overhead)."""
    key = ("nc", reps)
    if key in _cache:
        return _cache[key]
    nc = bacc.Bacc(None, target_bir_lowering=False, debug=False,
                   num_devices=NCORES)
    d = _declare_io(nc)
    with TileContext(nc) as tc:
        for _ in range(reps):
            _emit_once(nc, tc, d)
    nc.compile()
    _cache[key] = nc
    return nc


def _build():
    return build_program(1)


def make_in_maps(inputs):
    f = lambda name: np.ascontiguousarray(np.asarray(inputs[name], np.float32))
    x = f("input")
    yh = f("main_h")
    yc = f("main_c")
    mh = f("meta_h")
    mc = f("meta_c")
    t = int(np.asarray(inputs["task_index"]))
    wiH = f("weight_iH")[t]          # [4H, I]
    wHH = f("weight_HH")[t]          # [4H, H]
    wih = f("weight_ih")             # [4HH, I+H]
    whh = f("weight_hh")             # [4HH, HH]
    hzi, hzH, hzb = f("weight_hzi"), f("weight_hzH"), f("weight_hzb")
    dzi, dzH, bz = f("weight_dziH"), f("weight_dzHH"), f("weight_bzH")
    b_i, b_H = f("bias_i"), f("bias_H")
    bias, bhyp = f("bias"), f("bias_hyper")

    C = np.ascontiguousarray
    xT = C(x.T)
    yT = C(yh.T)
    mT = C(mh.T)
    ycT = C(yc.T)
    mcT = C(mc.T)
    hziT, hzHT, hzbT = C(hzi.T), C(hzH.T), C(hzb.T)
    biT = C(b_i[:, None])
    bHT = C(b_H[:, None])

    in_maps = []
    for k in range(NCORES):
        mrows = np.concatenate([g * HH + np.arange(k * MS, (k + 1) * MS)
                                for g in range(4)])
        jrows = np.concatenate([g * H + np.arange(k * HS, (k + 1) * HS)
                                for g in range(4)])
        in_maps.append({
            "xT": xT, "yT": yT, "mT": mT,
            "mcT": C(mcT[k * MS:(k + 1) * MS]),
            "ycT": C(ycT[k * HS:(k + 1) * HS]),
            "wihx": C(wih[mrows, :I].T),
            "wihh": C(wih[mrows, I:].T),
            "whh": C(whh[mrows].T),
            "hzi": hziT, "hzH": hzHT, "hzb": hzbT,
            "wiH": C(wiH[jrows].T),
            "wHH": C(wHH[jrows].T),
            "dzi": C(dzi[jrows].T),
            "dzH": C(dzH[jrows].T),
            "bz": C(bz[jrows].T),
            "bhyp": C(bhyp[mrows].reshape(4, MS).T),
            "bmain": C(bias[jrows].reshape(4, HS).T),
            "bi": biT, "bH": bHT,
        })
    return in_maps


def assemble(results):
    mh = np.concatenate([results[k]["o_mh"] for k in range(NCORES)], axis=0).T
    mc = np.concatenate([results[k]["o_mc"] for k in range(NCORES)], axis=0).T
    yh = np.concatenate([results[k]["o_yh"] for k in range(NCORES)], axis=0).T
    yc = np.concatenate([results[k]["o_yc"] for k in range(NCORES)], axis=0).T
    return ((np.ascontiguousarray(yh), np.ascontiguousarray(yc)),
            (np.ascontiguousarray(mh), np.ascontiguousarray(mc)))


def kernel(**inputs):
    nc = _build()
    in_maps = make_in_maps(inputs)
    res = run_bass_kernel_spmd(nc, in_maps, list(range(NCORES)))
    return assemble(res.results)
